# revision 1
# baseline (speedup 1.0000x reference)
"""3-layer GCN (GCNConv x3) on Trainium2, distributed across 8 NeuronCores.

Strategy (graph/data parallel, per the sharding hint):
  - Nodes are block-partitioned across the 8 cores; each core owns the
    destination side (scatter-add aggregation) for its node shard.
  - The tiny weight matrices are replicated; per layer each core computes
    g = dis * h for its shard, the shards are AllGathered into a shared HBM
    table G, and each core aggregates messages for its own nodes with
    dma_gather (256B rows from G) + dma_scatter_add (CCE-add, SBUF
    destination in the parity-split layout).
  - Symmetric normalization is folded per-node:
       (Ahat h)[c] = dis[c] * ( sum_{e->c} dis[r] h[r] + dis[c] h[c] )
    so no per-edge multiplies are needed on-device.
  - Scatter rounds have distinct destinations within each call (race-free
    CCE accumulation); high-degree columns spill to a second virtual level
    summed during readback. Rounds are split into NSUB destination ranges
    with disjoint output APs so sub-calls of one round never serialize.
"""
import sys
import numpy as np

sys.path.insert(0, "/opt/trn_rl_repo")

F = 64           # feature width (STATE == HID == 64)
NCORES = 8


class Plan:
    """Static (compile-time) layout shared by all cores, plus per-core data.

    Node layout: local node l = t*128 + p  (SBUF partition p, tile t).
    Global gather-table row of node (core k, local l) = k*sh + l.
    Scatter destination slot of (level, local l): g_s = level*T + t, range
    j = g_s // Q, s_local = g_s % Q, idx = s_local*128 + p (trash at
    s_local = Q). Buffer pair (A = even s_local, B = odd), group column =
    W//2 * j + s_local//2.
    """

    def __init__(self, n_nodes, edge_index, cap_min=24, row_chunk=32768):
        self.n_nodes = n_nodes
        self.row_chunk = row_chunk
        shard = (n_nodes + NCORES - 1) // NCORES
        sh = ((shard + 127) // 128) * 128
        self.shard, self.sh, self.t = shard, sh, sh // 128
        self.gtbl_rows = ((sh * NCORES + row_chunk - 1) // row_chunk) * row_chunk
        self.n_chunks = self.gtbl_rows // row_chunk

        row = np.asarray(edge_index[0], dtype=np.int64)
        col = np.asarray(edge_index[1], dtype=np.int64)
        deg = np.bincount(col, minlength=n_nodes).astype(np.float64) + 1.0
        self.dis = (1.0 / np.sqrt(deg)).astype(np.float32)

        tpos_row = (row // shard) * sh + (row % shard)
        dst_core = col // shard
        cloc = col % shard

        per_core = []
        maxdeg = 0
        for k in range(NCORES):
            m = dst_core == k
            r_k, c_k = tpos_row[m], cloc[m]
            o = np.argsort(c_k, kind="stable")
            cs = c_k[o]
            if cs.size:
                starts = np.r_[True, cs[1:] != cs[:-1]]
                run_starts = np.flatnonzero(starts)
                rid = np.cumsum(starts) - 1
                occ_s = np.arange(cs.size) - run_starts[rid]
                occ = np.empty_like(occ_s)
                occ[o] = occ_s
                maxdeg = max(maxdeg, int(occ_s.max()) + 1)
            else:
                occ = np.zeros(0, np.int64)
            per_core.append((r_k, c_k, occ))

        self.levels = 2
        self.cap = max(cap_min, (maxdeg + self.levels - 1) // self.levels)
        assert maxdeg <= self.cap * self.levels

        T = self.t
        tot_slots = self.levels * T
        for nsub in (4, 2, 1):
            if tot_slots % nsub == 0:
                self.nsub = nsub
                break
        self.q_slots = tot_slots // self.nsub                 # real slots/range
        self.trash_slots = 2 if self.q_slots % 2 == 0 else 1
        self.w_slots = self.q_slots + self.trash_slots        # even
        assert self.w_slots % 2 == 0
        self.grp_per_range = self.w_slots // 2
        self.agg_groups = self.grp_per_range * self.nsub
        assert (self.w_slots - 1) * 128 + 127 <= 32767

        NS, NC = self.nsub, self.n_chunks
        cnt = np.zeros((NCORES, self.cap, NS, NC), np.int64)
        for k, (r_k, c_k, occ) in enumerate(per_core):
            rnd = occ % self.cap
            lev = occ // self.cap
            g_s = lev * T + (c_k >> 7)
            rng = g_s // self.q_slots
            chk = r_k // row_chunk
            np.add.at(cnt[k], (rnd, rng, chk), 1)
        n_rjc = cnt.max(axis=0)
        n_rjc = np.maximum((n_rjc + 127) // 128 * 128, 128)
        self.n_rjc = n_rjc                                   # [cap, NS, NC]
        self.m_rj = n_rjc.sum(axis=2)
        self.n_r = self.m_rj.sum(axis=1)
        self.max_nr = int(self.n_r.max())
        self.tot_tok = int(self.n_r.sum())

        base_r = np.concatenate([[0], np.cumsum(self.n_r)])[:-1]
        off_rj = np.zeros((self.cap, NS), np.int64)
        off_rjc = np.zeros((self.cap, NS, NC), np.int64)
        for r in range(self.cap):
            o = 0
            for j in range(NS):
                off_rj[r, j] = o
                for c in range(NC):
                    off_rjc[r, j, c] = o
                    o += int(n_rjc[r, j, c])
        self.base_r, self.off_rj, self.off_rjc = base_r, off_rj, off_rjc

        self.gidx = []
        self.sidx = []
        for k, (r_k, c_k, occ) in enumerate(per_core):
            rnd = (occ % self.cap).astype(np.int64)
            lev = (occ // self.cap).astype(np.int64)
            tt = c_k >> 7
            pp = c_k & 127
            g_s = lev * T + tt
            s_local = g_s % self.q_slots
            rng = g_s // self.q_slots
            chk = r_k // row_chunk
            gflat = np.zeros(self.tot_tok, np.int64)
            sflat = np.full(self.tot_tok, self.q_slots * 128, np.int64)
            key = (rnd * NS + rng) * NC + chk
            order = np.argsort(key, kind="stable")
            ks = key[order]
            if ks.size:
                starts = np.r_[True, ks[1:] != ks[:-1]]
                run_starts = np.flatnonzero(starts)
                rid = np.cumsum(starts) - 1
                within = np.arange(ks.size) - run_starts[rid]
                rr = ks // (NS * NC)
                jj = (ks // NC) % NS
                cc = ks % NC
                pos = base_r[rr] + off_rjc[rr, jj, cc] + within
                e = order
                gflat[pos] = r_k[e] - cc * row_chunk
                sflat[pos] = s_local[e] * 128 + pp[e]
            self.gidx.append(self._wrap(gflat))
            self.sidx.append(self._wrap(sflat))

        self.gso = {}
        self.sso = {}
        off = 0
        for r in range(self.cap):
            for j in range(NS):
                for c in range(NC):
                    self.gso[(r, j, c)] = off
                    off += int(n_rjc[r, j, c]) // 16
        self.gslots = off
        off = 0
        for r in range(self.cap):
            for j in range(NS):
                self.sso[(r, j)] = off
                off += int(self.m_rj[r, j]) // 16
        self.sslots = off

    @staticmethod
    def _wrap(idx):
        n = idx.size
        a = idx.astype(np.int16).reshape(n // 16, 16).T
        return np.ascontiguousarray(np.tile(a, (8, 1)))

    def core_inputs(self, k, x, W1, b1, W2, b2, W3, b3):
        sh, shard, t = self.sh, self.shard, self.t
        xs = np.zeros((sh, F), np.float32)
        lo, hi = k * shard, min((k + 1) * shard, self.n_nodes)
        xs[: hi - lo] = x[lo:hi]
        ds = np.zeros(sh, np.float32)
        ds[: hi - lo] = self.dis[lo:hi]
        x_dev = xs.reshape(t, 128, F).transpose(1, 0, 2)   # node l = t*128+p
        d_dev = ds.reshape(t, 128).T
        return {
            "x": np.ascontiguousarray(x_dev.reshape(128, t * F)),
            "dis": np.ascontiguousarray(d_dev),
            "gidx": self.gidx[k],
            "sidx": self.sidx[k],
            "W1": np.asarray(W1, np.float32),
            "b1": np.asarray(b1, np.float32).reshape(F, 1),
            "W2": np.asarray(W2, np.float32),
            "b2": np.asarray(b2, np.float32).reshape(F, 1),
            "W3": np.asarray(W3, np.float32).reshape(F, 1),
            "b3": np.asarray(b3, np.float32).reshape(1, 1),
        }

    def assemble(self, outs):
        """outs: per core {'out': [sh]} with flat index == node local id."""
        res = np.zeros((self.n_nodes, 1), np.float32)
        for k in range(NCORES):
            o = np.asarray(outs[k]["out"]).reshape(self.sh)
            lo = k * self.shard
            hi = min(lo + self.shard, self.n_nodes)
            res[lo:hi, 0] = o[: hi - lo]
        return res


def build(plan, n_layers=3):
    import concourse.bacc as bacc
    import concourse.mybir as mybir
    import concourse.tile as tile
    from concourse.masks import make_identity

    f32 = mybir.dt.float32
    i16 = mybir.dt.int16
    T, SH = plan.t, plan.sh
    CAP, NC, NS = plan.cap, plan.n_chunks, plan.nsub
    MAXNR = plan.max_nr
    GPR = plan.grp_per_range

    nc = bacc.Bacc("TRN2", target_bir_lowering=False, debug=False,
                   num_devices=NCORES, num_swdge_queues=4)

    x_t = nc.dram_tensor("x", [128, T * F], f32, kind="ExternalInput")
    dis_t = nc.dram_tensor("dis", [128, T], f32, kind="ExternalInput")
    gidx_t = nc.dram_tensor("gidx", [128, plan.gslots], i16, kind="ExternalInput")
    sidx_t = nc.dram_tensor("sidx", [128, plan.sslots], i16, kind="ExternalInput")
    Ws = {}
    for nm, shape in [("W1", [F, F]), ("b1", [F, 1]), ("W2", [F, F]),
                      ("b2", [F, 1]), ("W3", [F, 1]), ("b3", [1, 1])]:
        Ws[nm] = nc.dram_tensor(nm, shape, f32, kind="ExternalInput")
    out_t = nc.dram_tensor("out", [SH], f32, kind="ExternalOutput")

    g_dram = nc.dram_tensor("g_bounce", [SH * F], f32, kind="Internal")
    G = nc.dram_tensor("G_table", [plan.gtbl_rows, F], f32, kind="Internal",
                       addr_space="Shared")
    rg = [list(range(NCORES))]

    with tile.TileContext(nc) as tc:
        with tc.tile_pool(name="const", bufs=1) as cpool, \
             tc.tile_pool(name="state", bufs=1) as spool, \
             tc.tile_pool(name="agg", bufs=1) as apool, \
             tc.tile_pool(name="msg", bufs=2) as mpool, \
             tc.tile_pool(name="idx", bufs=2) as ipool, \
             tc.tile_pool(name="fm", bufs=2) as fpool, \
             tc.tile_pool(name="psum", bufs=2, space="PSUM") as ppool:

            ident = cpool.tile([128, 128], f32)
            make_identity(nc, ident[:])
            dis_s = cpool.tile([128, T], f32)
            nc.sync.dma_start(dis_s[:], dis_t[:])
            wsb = {}
            for nm in ("W1", "W2", "W3", "b1", "b2", "b3"):
                wsb[nm] = cpool.tile(list(Ws[nm].shape), f32, name=f"sb_{nm}")
                nc.sync.dma_start(wsb[nm][:], Ws[nm][:])

            dis_b = dis_s[:].unsqueeze(-1).broadcast_to([128, T, F])

            aggA = apool.tile([128, plan.agg_groups, F], f32, tag="aggA")
            aggB = apool.tile([128, plan.agg_groups, F], f32, tag="aggB")

            g = spool.tile([128, T, F], f32, tag="g")
            tmp = spool.tile([128, T, F], f32, tag="acc")
            nc.sync.dma_start(tmp[:], x_t[:].rearrange("p (t f) -> p t f", f=F))
            nc.vector.tensor_tensor(g[:], tmp[:], dis_b, mybir.AluOpType.mult)

            # strided view: DRAM row l = t*128+p  <->  g[p, t, :]
            g_dram_v = g_dram[:].rearrange("(t p f) -> p t f", p=128, f=F)

            def agg_src(lev, t):
                g_s = lev * T + t
                j, s = divmod(g_s, plan.q_slots)
                buf = aggA if s % 2 == 0 else aggB
                return buf, GPR * j + s // 2

            for layer in range(n_layers):
                nc.sync.dma_start(g_dram_v, g[:])
                nc.gpsimd.collective_compute(
                    "AllGather", mybir.AluOpType.bypass,
                    replica_groups=rg,
                    ins=[g_dram[:]],
                    outs=[G[0:SH * NCORES, :].rearrange("r f -> (r f)")],
                )
                nc.vector.memset(aggA[:], 0.0)
                nc.vector.memset(aggB[:], 0.0)

                for r in range(CAP):
                    n_r = int(plan.n_r[r])
                    gi = ipool.tile([128, MAXNR // 16], i16, tag="gi")
                    si = ipool.tile([128, MAXNR // 16], i16, tag="si")
                    g0 = plan.gso[(r, 0, 0)]
                    s0 = plan.sso[(r, 0)]
                    nc.sync.dma_start(gi[:, : n_r // 16],
                                      gidx_t[:, g0:g0 + n_r // 16])
                    nc.sync.dma_start(si[:, : n_r // 16],
                                      sidx_t[:, s0:s0 + n_r // 16])
                    buf = mpool.tile([128, MAXNR // 128, F], f32, tag="msg")
                    for j in range(NS):
                        oj = int(plan.off_rj[r, j])
                        for c in range(NC):
                            n_rjc = int(plan.n_rjc[r, j, c])
                            oc = int(plan.off_rjc[r, j, c])
                            go = plan.gso[(r, j, c)] - g0
                            nc.gpsimd.dma_gather(
                                buf[:, oc // 128:(oc + n_rjc) // 128, :],
                                G[c * plan.row_chunk:(c + 1) * plan.row_chunk, :],
                                gi[:, go:go + n_rjc // 16],
                                n_rjc, n_rjc, F,
                                queue_num=2 * (j % 2),
                                single_packet=bool(n_rjc <= 1024))
                        m_rj = int(plan.m_rj[r, j])
                        so = plan.sso[(r, j)] - s0
                        nc.gpsimd.dma_scatter_add(
                            aggA[:, GPR * j:GPR * (j + 1), :],
                            buf[:, oj // 128:(oj + m_rj) // 128, :],
                            si[:, so:so + m_rj // 16],
                            m_rj, m_rj, F,
                            sbuf_tokens_per_rank=128,
                            parity_reg=0,
                            out_ap_other=aggB[:, GPR * j:GPR * (j + 1), :],
                            queue_num=1 + 2 * (j % 2))

                acc = spool.tile([128, T, F], f32, tag="acc")
                for t in range(T):
                    b0, g0i = agg_src(0, t)
                    b1v, g1i = agg_src(1, t)
                    nc.vector.tensor_tensor(acc[:, t, :], b0[:, g0i, :],
                                            b1v[:, g1i, :], mybir.AluOpType.add)
                nc.vector.tensor_tensor(acc[:], acc[:], g[:], mybir.AluOpType.add)
                nc.vector.tensor_tensor(acc[:], acc[:], dis_b, mybir.AluOpType.mult)

                if layer < n_layers - 1:
                    W, b = wsb[f"W{layer + 1}"], wsb[f"b{layer + 1}"]
                    g2 = spool.tile([128, T, F], f32, tag="g")
                    for t0 in range(0, T, 4):
                        nt = min(4, T - t0)
                        fm = fpool.tile([F, 4 * 128], f32, tag="fm")
                        for j in range(nt):
                            pt = ppool.tile([F, 128], f32, space="PSUM", tag="pt")
                            nc.tensor.transpose(pt[:], acc[:, t0 + j, :], ident[:])
                            nc.vector.tensor_copy(fm[:, j * 128:(j + 1) * 128], pt[:])
                        mm = ppool.tile([F, 4 * 128], f32, space="PSUM", tag="mm")
                        nc.tensor.matmul(mm[:, : nt * 128], W[:], fm[:, : nt * 128],
                                         start=True, stop=True)
                        hfm = fpool.tile([F, 4 * 128], f32, tag="hfm")
                        nc.scalar.activation(hfm[:, : nt * 128], mm[:, : nt * 128],
                                             mybir.ActivationFunctionType.Relu,
                                             bias=b[:, :1])
                        for j in range(nt):
                            pt2 = ppool.tile([128, F], f32, space="PSUM", tag="pt2")
                            nc.tensor.transpose(
                                pt2[:], hfm[:, j * 128:(j + 1) * 128], ident[:F, :F])
                            nc.vector.tensor_scalar_mul(
                                g2[:, t0 + j, :], pt2[:], dis_s[:, t0 + j:t0 + j + 1])
                    g = g2
                else:
                    W3, b3 = wsb["W3"], wsb["b3"]
                    for t0 in range(0, T, 4):
                        nt = min(4, T - t0)
                        fm = fpool.tile([F, 4 * 128], f32, tag="fm")
                        for j in range(nt):
                            pt = ppool.tile([F, 128], f32, space="PSUM", tag="pt")
                            nc.tensor.transpose(pt[:], acc[:, t0 + j, :], ident[:])
                            nc.vector.tensor_copy(fm[:, j * 128:(j + 1) * 128], pt[:])
                        mm3 = ppool.tile([1, 4 * 128], f32, space="PSUM", tag="mm")
                        nc.tensor.matmul(mm3[:, : nt * 128], W3[:], fm[:, : nt * 128],
                                         start=True, stop=True)
                        ofm = fpool.tile([1, 4 * 128], f32, tag="ofm")
                        nc.vector.tensor_scalar_add(
                            ofm[:, : nt * 128], mm3[:, : nt * 128], b3[:, :1])
                        nc.sync.dma_start(
                            out_t[t0 * 128:(t0 + nt) * 128]
                            .rearrange("(a x) -> a x", a=1),
                            ofm[:, : nt * 128])

    nc.compile()
    return nc


def kernel(**inputs):
    from concourse import bass2jax

    x = np.asarray(inputs["x"], np.float32)
    edge_index = np.asarray(inputs["edge_index"])
    plan = Plan(x.shape[0], edge_index)
    nc = build(plan)
    in_maps = [plan.core_inputs(k, x, inputs["W1"], inputs["b1"], inputs["W2"],
                                inputs["b2"], inputs["W3"], inputs["b3"])
               for k in range(NCORES)]
    results = bass2jax.run_bass_via_pjrt(nc, in_maps, n_cores=NCORES)
    return plan.assemble(results)



# revision 76
# speedup vs baseline: 4.1989x; 4.1989x over previous
"""3-layer GCN (GCNConv x3) on Trainium2, distributed across 8 NeuronCores.

Strategy (graph/data parallel, per the sharding hint):
  - Nodes are block-partitioned across the 8 cores; each core owns the
    destination side (scatter-add aggregation) for its node shard.
  - The tiny weight matrices are replicated; per layer each core computes
    g = dis * h for its shard, the shards are AllGathered into a shared HBM
    table G, and each core aggregates messages for its own nodes with
    dma_gather (256B rows from G) + dma_scatter_add (CCE-add, SBUF
    destination in the parity-split layout).
  - Symmetric normalization is folded per-node:
       (Ahat h)[c] = dis[c] * sum_{e->c, self} dis[r] h[r]
    (self-loops are regular tokens) so no per-edge multiplies on-device.
  - Scatter rounds have distinct destination slots within each call
    (race-free CCE accumulation); a per-dest phase staggers rounds so each
    carries ~tot/cap tokens, and each round issues one scatter per
    dest-tile half (disjoint slot windows, and each call stays under the
    SWDGE descriptor-ring capacity) plus NC gathers (one per 32768-row
    int16 index window).  High-degree dests spill to a second virtual
    level; parity == level, so the readback is two contiguous A+B adds.
  - The LAST conv has a 1-wide output, so h@W3 is computed during the
    second layer's matmul tail and only the per-node scalars are exchanged
    (dense AllGather, 64x less traffic, expanded into column 0 of the
    table rows); the aggregation gathers the same 256B rows but scatters
    4B tokens.
  - Index tables are uploaded 16-wide (the native wrap) and replicated to
    the 128-partition layout once on device; x is uploaded in bf16.
"""
import sys
import numpy as np

sys.path.insert(0, "/opt/trn_rl_repo")

F = 64           # feature width (STATE == HID == 64)
NCORES = 8


class Plan:
    """Static (compile-time) layout shared by all cores, plus per-core data.

    Node layout: local node l = t*128 + p  (SBUF partition p, tile t).
    Global gather-table row of node (core k, local l) = k*sh + l.
    Scatter range of local l: rng = t // half (dest-tile halves); slot
    within the range: s_local = (t % half)*2 + lev, scatter index =
    s_local*128 + p (trash at s_local = q_slots, one per parity).  The
    parity of s_local equals the level, so buffer A collects level 0 and
    buffer B level 1, both at pair-column gpr*rng + (t % half), and the
    readback is acc[:, rng-half] = A[cols] + B[cols] per range.
    """

    def __init__(self, n_nodes, edge_index, row_chunk=32768):
        self.n_nodes = n_nodes
        self.row_chunk = row_chunk
        shard = (n_nodes + NCORES - 1) // NCORES
        sh = ((shard + 127) // 128) * 128
        self.shard, self.sh, self.t = shard, sh, sh // 128
        self.gtbl_rows = ((sh * NCORES + row_chunk - 1) // row_chunk) * row_chunk
        self.n_chunks = self.gtbl_rows // row_chunk

        loops = np.arange(n_nodes, dtype=np.int64)
        row = np.concatenate([np.asarray(edge_index[0], dtype=np.int64), loops])
        col = np.concatenate([np.asarray(edge_index[1], dtype=np.int64), loops])
        deg = np.bincount(col, minlength=n_nodes).astype(np.float64)
        self.dis = (1.0 / np.sqrt(deg)).astype(np.float32)

        tpos_row = (row // shard) * sh + (row % shard)
        dst_core = col // shard
        cloc = col % shard

        per_core = []
        maxdeg = 0
        for k in range(NCORES):
            m = dst_core == k
            r_k, c_k = tpos_row[m], cloc[m]
            o = np.argsort(c_k, kind="stable")
            cs = c_k[o]
            if cs.size:
                starts = np.r_[True, cs[1:] != cs[:-1]]
                run_starts = np.flatnonzero(starts)
                rid = np.cumsum(starts) - 1
                occ_s = np.arange(cs.size) - run_starts[rid]
                occ = np.empty_like(occ_s)
                occ[o] = occ_s
                maxdeg = max(maxdeg, int(occ_s.max()) + 1)
            else:
                occ = np.zeros(0, np.int64)
            per_core.append((r_k, c_k, occ))

        T = self.t
        assert T % 2 == 0
        self.levels = 2
        self.half = T // 2              # dest tiles per scatter range
        self.q_slots = self.levels * self.half   # slots per range
        self.w_slots = self.q_slots + 2          # + one trash slot per parity
        self.gpr = self.w_slots // 2    # buffer-pair columns per range
        self.agg_groups = self.gpr * self.levels
        assert (self.w_slots - 1) * 128 + 127 <= 32767

        self.cap = max((maxdeg + self.levels - 1) // self.levels, 1)
        assert maxdeg <= self.cap * self.levels

        NC = self.n_chunks
        NL = 2                          # scatter ranges (dest-tile halves)
        cnt = np.zeros((NCORES, self.cap, NL, NC), np.int64)
        for k, (r_k, c_k, occ) in enumerate(per_core):
            # per-dest phase staggers the rounds so each round carries
            # ~tot/cap tokens instead of round 0 taking every dest
            rnd = (occ % self.cap + c_k % self.cap) % self.cap
            rng = (c_k >> 7) // self.half
            chk = r_k // row_chunk
            np.add.at(cnt[k], (rnd, rng, chk), 1)
        n_act = cnt.max(axis=0)
        n_rlc = np.maximum((n_act + 127) // 128 * 128, 128)
        # real tokens per bucket (worst core), quantized up to 16: the
        # gather's count register lets the ucode skip the trailing pad
        # (indices there are -1); SPMD-shared so it must be the max.
        self.n_act = np.minimum(np.maximum((n_act + 15) // 16 * 16, 16), n_rlc)
        self.n_rlc = n_rlc                                   # [cap, NL, NC]
        self.n_rl = self.n_rlc.sum(axis=2)                   # [cap, NL]
        self.max_nrl = int(self.n_rl.max())
        self.tot_tok = int(self.n_rl.sum())
        # one scatter call per (round, range); keep it under the SWDGE
        # descriptor-ring capacity (~1024 descs at num_idxs/8 per desc)
        assert self.max_nrl <= 8064, self.max_nrl

        base = np.concatenate([[0], np.cumsum(self.n_rl.reshape(-1))])[:-1]
        base_rl = base.reshape(self.cap, NL)
        off_rlc = np.zeros((self.cap, NL, NC), np.int64)
        for r in range(self.cap):
            for l in range(NL):
                o = 0
                for c in range(NC):
                    off_rlc[r, l, c] = o
                    o += int(n_rlc[r, l, c])
        self.base_rl, self.off_rlc = base_rl, off_rlc

        self.gidx = []
        self.sidx = []
        for k, (r_k, c_k, occ) in enumerate(per_core):
            rnd = ((occ % self.cap + c_k % self.cap) % self.cap).astype(np.int64)
            lev = (occ // self.cap).astype(np.int64)
            tt = c_k >> 7
            pp = c_k & 127
            rng = tt // self.half
            # parity == level, pair-column == tile-within-range
            s_local = (tt % self.half) * 2 + lev
            chk = r_k // row_chunk
            gflat = np.full(self.tot_tok, -1, np.int64)
            sflat = np.full(self.tot_tok, self.q_slots * 128, np.int64)
            key = (rnd * NL + rng) * NC + chk
            order = np.argsort(key, kind="stable")
            ks = key[order]
            if ks.size:
                starts = np.r_[True, ks[1:] != ks[:-1]]
                run_starts = np.flatnonzero(starts)
                rid = np.cumsum(starts) - 1
                within = np.arange(ks.size) - run_starts[rid]
                rr = ks // (NL * NC)
                ll = (ks // NC) % NL
                cc = ks % NC
                pos = base_rl[rr, ll] + off_rlc[rr, ll, cc] + within
                e = order
                gflat[pos] = r_k[e] - cc * row_chunk
                sflat[pos] = s_local[e] * 128 + pp[e]
            # dummy row-0 gathers up to the shared count register value,
            # skipped (-1) beyond it
            for r in range(self.cap):
                for l in range(NL):
                    for c in range(NC):
                        s0 = base_rl[r, l] + off_rlc[r, l, c]
                        seg = gflat[s0:s0 + int(self.n_act[r, l, c])]
                        seg[seg < 0] = 0
            self.gidx.append(self._wrap(gflat))
            self.sidx.append(self._wrap(sflat))

        # int16-slot offsets into the flat [16, slots] index tables
        self.gso = {}
        off = 0
        for r in range(self.cap):
            for l in range(NL):
                for c in range(NC):
                    self.gso[(r, l, c)] = off
                    off += int(n_rlc[r, l, c]) // 16
        self.gslots = off
        self.sso = {}
        off = 0
        for r in range(self.cap):
            for l in range(NL):
                self.sso[(r, l)] = off
                off += int(self.n_rl[r, l]) // 16
        self.sslots = off

    @staticmethod
    def _wrap(idx):
        n = idx.size
        a = idx.astype(np.int16).reshape(n // 16, 16).T
        return np.ascontiguousarray(a)          # [16, n // 16]

    def core_inputs(self, k, x, W1, b1, W2, b2, W3, b3):
        import ml_dtypes
        sh, shard, t = self.sh, self.shard, self.t
        xs = np.zeros((sh, F), np.float32)
        lo, hi = k * shard, min((k + 1) * shard, self.n_nodes)
        xs[: hi - lo] = x[lo:hi]
        ds = np.zeros(sh, np.float32)
        ds[: hi - lo] = self.dis[lo:hi]
        x_dev = xs.reshape(t, 128, F).transpose(1, 0, 2)   # node l = t*128+p
        d_dev = ds.reshape(t, 128).T
        return {
            "x": np.ascontiguousarray(
                x_dev.reshape(128, t * F).astype(ml_dtypes.bfloat16)),
            "dis": np.ascontiguousarray(d_dev),
            "dis_flat": np.ascontiguousarray(ds.reshape(1, sh)),
            "gidx": self.gidx[k],
            "sidx": self.sidx[k],
            "W1": np.asarray(W1, np.float32),
            "b1": np.asarray(b1, np.float32).reshape(F, 1),
            "W2": np.asarray(W2, np.float32),
            "b2": np.asarray(b2, np.float32).reshape(F, 1),
            "W3": np.asarray(W3, np.float32).reshape(F, 1),
            "b3": np.asarray(b3, np.float32).reshape(1, 1),
        }

    def assemble(self, outs, b3=0.0):
        """outs: per core {'out': [sh]} with flat index == node local id."""
        res = np.zeros((self.n_nodes, 1), np.float32)
        for k in range(NCORES):
            o = np.asarray(outs[k]["out"]).reshape(self.sh)
            lo = k * self.shard
            hi = min(lo + self.shard, self.n_nodes)
            res[lo:hi, 0] = o[: hi - lo]
        return res + np.float32(b3).reshape(1, 1)


def build(plan, n_queues=4, scalar_l3=True):
    import concourse.bacc as bacc
    import concourse.mybir as mybir
    import concourse.tile as tile
    from concourse.masks import make_identity

    f32 = mybir.dt.float32
    bf16 = mybir.dt.bfloat16
    i16 = mybir.dt.int16
    T, SH = plan.t, plan.sh
    CAP, NC, NL = plan.cap, plan.n_chunks, plan.levels
    MAXNR = plan.max_nrl
    GPR = plan.gpr                     # pair-columns per range
    AGG = plan.agg_groups              # total pair-columns (NL * GPR)
    H = T // 2                         # even (= odd) tile count per range

    nc = bacc.Bacc("TRN2", target_bir_lowering=False, debug=False,
                   num_devices=NCORES, num_swdge_queues=n_queues)

    x_t = nc.dram_tensor("x", [128, T * F], bf16, kind="ExternalInput")
    dis_t = nc.dram_tensor("dis", [128, T], f32, kind="ExternalInput")
    disf_t = nc.dram_tensor("dis_flat", [1, SH], f32, kind="ExternalInput")
    gidx_t = nc.dram_tensor("gidx", [16, plan.gslots], i16, kind="ExternalInput")
    sidx_t = nc.dram_tensor("sidx", [16, plan.sslots], i16, kind="ExternalInput")
    Ws = {}
    for nm, shape in [("W1", [F, F]), ("b1", [F, 1]), ("W2", [F, F]),
                      ("b2", [F, 1]), ("W3", [F, 1]), ("b3", [1, 1])]:
        Ws[nm] = nc.dram_tensor(nm, shape, f32, kind="ExternalInput")
    out_t = nc.dram_tensor("out", [SH], f32, kind="ExternalOutput")

    gidx_r = nc.dram_tensor("gidx_rep", [128, plan.gslots], i16, kind="Internal")
    sidx_r = nc.dram_tensor("sidx_rep", [128, plan.sslots], i16, kind="Internal")
    g_dram = nc.dram_tensor("g_bounce", [SH * F], f32, kind="Internal")
    t3_dram = nc.dram_tensor("t3_bounce", [SH], f32, kind="Internal")
    T3d = nc.dram_tensor("T3_dense", [SH * NCORES], f32, kind="Internal",
                         addr_space="Shared")
    G = nc.dram_tensor("G_table", [plan.gtbl_rows, F], f32, kind="Internal",
                       addr_space="Shared")
    rg = [list(range(NCORES))]

    with tile.TileContext(nc) as tc:
        with tc.tile_pool(name="const", bufs=1) as cpool, \
             tc.tile_pool(name="state", bufs=1) as spool, \
             tc.tile_pool(name="agg", bufs=1) as apool, \
             tc.tile_pool(name="msg", bufs=4) as mpool, \
             tc.tile_pool(name="idx", bufs=4) as ipool, \
             tc.tile_pool(name="fm", bufs=2) as fpool, \
             tc.tile_pool(name="psum", bufs=2, space="PSUM") as ppool:

            # replicate the 16-wide index uploads to the 128-partition wrap
            # (on the otherwise-idle Activation engine so the SP-engine
            # bounce/collective chain is not queued behind them)
            for k in range(NCORES):
                nc.scalar.dma_start(gidx_r[16 * k:16 * (k + 1), :], gidx_t[:])
                nc.scalar.dma_start(sidx_r[16 * k:16 * (k + 1), :], sidx_t[:])

            ident = cpool.tile([128, 128], f32)
            make_identity(nc, ident[:])
            dis_s = cpool.tile([128, T], f32)
            nc.sync.dma_start(dis_s[:], dis_t[:])
            wsb = {}
            for nm in ("W1", "W2", "W3", "b1", "b2", "b3"):
                wsb[nm] = cpool.tile(list(Ws[nm].shape), f32, name=f"sb_{nm}")
                nc.sync.dma_start(wsb[nm][:], Ws[nm][:])

            dis_b = dis_s[:].unsqueeze(-1).broadcast_to([128, T, F])

            aggA = apool.tile([128, AGG, F], f32, tag="aggA")
            aggB = apool.tile([128, AGG, F], f32, tag="aggB")
            agg3A = apool.tile([128, AGG, 1], f32, tag="agg3A")
            agg3B = apool.tile([128, AGG, 1], f32, tag="agg3B")

            qctr = [0]

            def next_q():
                q = qctr[0] % n_queues
                qctr[0] += 1
                return q

            # the gather/scatter token counts are 128-quantized, so a few
            # shared registers replace one RegisterMove per call (~30us of
            # Pool-engine time per layer otherwise)
            reg_cache = {}

            def cached_reg(v):
                if v not in reg_cache:
                    reg_cache[v] = nc.gpsimd.to_reg(v)
                return reg_cache[v]

            xb = mpool.tile([128, T, F], bf16, tag="msg")
            nc.sync.dma_start(xb[:], x_t[:].rearrange("p (t f) -> p t f", f=F))
            g = spool.tile([128, T, F], f32, tag="g")
            tmp = spool.tile([128, T, F], f32, tag="acc")
            nc.vector.tensor_copy(tmp[:], xb[:])
            nc.vector.tensor_tensor(g[:], tmp[:], dis_b, mybir.AluOpType.mult)

            # strided view: DRAM row l = t*128+p  <->  g[p, t, :]
            g_dram_v = g_dram[:].rearrange("(t p f) -> p t f", p=128, f=F)

            def do_rounds(scalar):
                for r in range(CAP):
                    for l in range(NL):
                        n_rl = int(plan.n_rl[r, l])
                        gi = ipool.tile([128, MAXNR // 16], i16, tag="gi")
                        si = ipool.tile([128, MAXNR // 16], i16, tag="si")
                        g0 = plan.gso[(r, l, 0)]
                        s0 = plan.sso[(r, l)]
                        nc.sync.dma_start(gi[:, : n_rl // 16],
                                          gidx_r[:, g0:g0 + n_rl // 16])
                        nc.sync.dma_start(si[:, : n_rl // 16],
                                          sidx_r[:, s0:s0 + n_rl // 16])
                        buf = mpool.tile([128, MAXNR // 128, F], f32, tag="msg")
                        for c in range(NC):
                            n_rlc = int(plan.n_rlc[r, l, c])
                            oc = int(plan.off_rlc[r, l, c])
                            go = plan.gso[(r, l, c)] - g0
                            hi = min((c + 1) * plan.row_chunk, SH * NCORES)
                            nc.gpsimd.dma_gather(
                                buf[:, oc // 128:(oc + n_rlc) // 128, :],
                                G[c * plan.row_chunk:hi, :],
                                gi[:, go:go + n_rlc // 16],
                                n_rlc, cached_reg(int(plan.n_act[r, l, c])), F,
                                queue_num=next_q(),
                                single_packet=bool(n_rlc <= 1024))
                        if scalar:
                            v = mpool.tile([128, MAXNR // 128, 1], f32,
                                           tag="vmsg")
                            nc.vector.tensor_copy(v[:, : n_rl // 128, 0:1],
                                                  buf[:, : n_rl // 128, 0:1])
                            nc.gpsimd.dma_scatter_add(
                                agg3A[:, GPR * l:GPR * (l + 1), :],
                                v[:, : n_rl // 128, :],
                                si[:, : n_rl // 16],
                                n_rl, cached_reg(n_rl), 1,
                                sbuf_tokens_per_rank=128,
                                parity_reg=cached_reg(0),
                                out_ap_other=agg3B[:, GPR * l:GPR * (l + 1), :],
                                queue_num=next_q())
                        else:
                            nc.gpsimd.dma_scatter_add(
                                aggA[:, GPR * l:GPR * (l + 1), :],
                                buf[:, : n_rl // 128, :],
                                si[:, : n_rl // 16],
                                n_rl, cached_reg(n_rl), F,
                                sbuf_tokens_per_rank=128,
                                parity_reg=cached_reg(0),
                                out_ap_other=aggB[:, GPR * l:GPR * (l + 1), :],
                                queue_num=next_q())

            n_floop = 2 if scalar_l3 else 3
            for layer in range(n_floop):
                if layer == 0:
                    nc.sync.dma_start(g_dram_v, g[:])
                # (later layers: the previous tail already wrote g_dram
                # block by block, so the collective can start immediately)
                nc.gpsimd.collective_compute(
                    "AllGather", mybir.AluOpType.bypass,
                    replica_groups=rg,
                    ins=[g_dram[:]],
                    outs=[G[0:SH * NCORES, :].rearrange("r f -> (r f)")],
                )
                nc.vector.memset(aggA[:], 0.0)
                nc.vector.memset(aggB[:], 0.0)

                do_rounds(scalar=False)

                # acc[tile j*H+u] = (A[range j, col u] + B[range j, col u])*dis,
                # per range so each half unblocks its matmul blocks early
                acc = spool.tile([128, T, F], f32, tag="acc")
                for j in range(NL):
                    sl = slice(j * H, (j + 1) * H)
                    nc.vector.tensor_tensor(
                        acc[:, sl, :],
                        aggA[:, GPR * j:GPR * j + H, :],
                        aggB[:, GPR * j:GPR * j + H, :], mybir.AluOpType.add)
                    nc.vector.tensor_tensor(acc[:, sl, :], acc[:, sl, :],
                                            dis_b[:, sl, :], mybir.AluOpType.mult)

                if not scalar_l3 and layer == 2:
                    W3, b3s = wsb["W3"], wsb["b3"]
                    for t0 in range(0, T, 4):
                        nt = min(4, T - t0)
                        fm = fpool.tile([F, 4 * 128], f32, tag="fm")
                        for j in range(nt):
                            pt = ppool.tile([F, 128], f32, space="PSUM", tag="pt")
                            nc.tensor.transpose(pt[:], acc[:, t0 + j, :], ident[:])
                            nc.vector.tensor_copy(fm[:, j * 128:(j + 1) * 128], pt[:])
                        mm3 = ppool.tile([1, 4 * 128], f32, space="PSUM", tag="mm3")
                        nc.tensor.matmul(mm3[:, : nt * 128], W3[:], fm[:, : nt * 128],
                                         start=True, stop=True)
                        ofm = fpool.tile([1, 4 * 128], f32, tag="tofm")
                        nc.vector.tensor_scalar_add(
                            ofm[:, : nt * 128], mm3[:, : nt * 128], b3s[:, :1])
                        nc.sync.dma_start(
                            out_t[t0 * 128:(t0 + nt) * 128]
                            .rearrange("(a x) -> a x", a=1),
                            ofm[:, : nt * 128])
                    continue
                W, b = wsb[f"W{layer + 1}"], wsb[f"b{layer + 1}"]
                last = layer == 1 and scalar_l3
                if last:
                    W3 = wsb["W3"]
                for t0 in range(0, T, 4):
                    nt = min(4, T - t0)
                    ptA = ppool.tile([F, 4 * 128], f32, space="PSUM", tag="pt")
                    for j in range(nt):
                        nc.tensor.transpose(ptA[:, j * 128:(j + 1) * 128],
                                            acc[:, t0 + j, :], ident[:])
                    fm = fpool.tile([F, 4 * 128], f32, tag="fm")
                    nc.vector.tensor_copy(fm[:, : nt * 128], ptA[:, : nt * 128])
                    mm = ppool.tile([F, 4 * 128], f32, space="PSUM", tag="mm")
                    nc.tensor.matmul(mm[:, : nt * 128], W[:], fm[:, : nt * 128],
                                     start=True, stop=True)
                    hfm = fpool.tile([F, 4 * 128], f32, tag="hfm")
                    nc.scalar.activation(hfm[:, : nt * 128], mm[:, : nt * 128],
                                         mybir.ActivationFunctionType.Relu,
                                         bias=b[:, :1])
                    if not last:
                        # transpose back, scale by next-layer dis, and ship
                        # the block straight to the bounce buffer
                        pt2 = ppool.tile([128, 4 * F], f32, space="PSUM",
                                         tag="pt2")
                        for j in range(nt):
                            nc.tensor.transpose(
                                pt2[:, j * F:(j + 1) * F],
                                hfm[:, j * 128:(j + 1) * 128], ident[:F, :F])
                        gblk = fpool.tile([128, 4 * F], f32, tag="gblk")
                        gblk_v = gblk[:].rearrange("p (t f) -> p t f", f=F)
                        pt2_v = pt2[:].rearrange("p (t f) -> p t f", f=F)
                        nc.vector.tensor_tensor(
                            gblk_v[:, : nt, :], pt2_v[:, : nt, :],
                            dis_b[:, t0:t0 + nt, :], mybir.AluOpType.mult)
                        nc.sync.dma_start(g_dram_v[:, t0:t0 + nt, :],
                                          gblk_v[:, : nt, :])
                    else:
                        # z = h2 @ W3 (scalar per node), t = dis * z,
                        # written straight into column 0 of the table rows
                        mm3 = ppool.tile([1, 4 * 128], f32, space="PSUM", tag="mm3")
                        nc.tensor.matmul(mm3[:, : nt * 128], W3[:],
                                         hfm[:, : nt * 128], start=True, stop=True)
                        df = fpool.tile([1, 4 * 128], f32, tag="df")
                        nc.sync.dma_start(
                            df[:, : nt * 128],
                            disf_t[:, t0 * 128:t0 * 128 + nt * 128])
                        tofm = fpool.tile([1, 4 * 128], f32, tag="tofm")
                        nc.vector.tensor_tensor(
                            tofm[:, : nt * 128], mm3[:, : nt * 128],
                            df[:, : nt * 128], mybir.AluOpType.mult)
                        nc.sync.dma_start(
                            t3_dram[t0 * 128:t0 * 128 + nt * 128]
                            .rearrange("(a x) -> a x", a=1),
                            tofm[:, : nt * 128])

            # third conv: exchange the per-node scalars densely (64x less
            # traffic than the F-wide table), expand into column 0 of the
            # table rows, gather the same 256B rows, scatter 4B tokens.
            if scalar_l3:
                nc.gpsimd.collective_compute(
                    "AllGather", mybir.AluOpType.bypass,
                    replica_groups=rg,
                    ins=[t3_dram[:]],
                    outs=[T3d[:]],
                )
                tsb = spool.tile([128, NCORES * T], f32, tag="tsb")
                nc.sync.dma_start(tsb[:], T3d[:].rearrange("(c p) -> p c", p=128))
                nc.sync.dma_start(
                    G[0:SH * NCORES, :]
                    .rearrange("(c p) f -> p c f", p=128)[:, :, 0:1],
                    tsb[:].unsqueeze(-1))
                nc.vector.memset(agg3A[:], 0.0)
                nc.vector.memset(agg3B[:], 0.0)

                do_rounds(scalar=True)

                res = spool.tile([128, T], f32, tag="res")
                for j in range(NL):
                    nc.vector.tensor_tensor(
                        res[:, j * H:(j + 1) * H],
                        agg3A[:, GPR * j:GPR * j + H, 0],
                        agg3B[:, GPR * j:GPR * j + H, 0], mybir.AluOpType.add)
                nc.vector.tensor_tensor(res[:], res[:], dis_s[:],
                                        mybir.AluOpType.mult)
                nc.sync.dma_start(out_t[:].rearrange("(t p) -> p t", p=128),
                                  res[:])

    nc.compile()
    return nc


def kernel(**inputs):
    from concourse import bass2jax

    x = np.asarray(inputs["x"], np.float32)
    edge_index = np.asarray(inputs["edge_index"])
    plan = Plan(x.shape[0], edge_index)
    nc = build(plan)
    in_maps = [plan.core_inputs(k, x, inputs["W1"], inputs["b1"], inputs["W2"],
                                inputs["b2"], inputs["W3"], inputs["b3"])
               for k in range(NCORES)]
    results = bass2jax.run_bass_via_pjrt(nc, in_maps, n_cores=NCORES)
    return plan.assemble(results, b3=np.asarray(inputs["b3"]).reshape(-1)[0])


# revision 83
# speedup vs baseline: 4.3860x; 1.0446x over previous
"""3-layer GCN (GCNConv x3) on Trainium2, distributed across 8 NeuronCores.

Strategy (graph/data parallel, per the sharding hint):
  - Nodes are block-partitioned across the 8 cores; each core owns the
    destination side (scatter-add aggregation) for its node shard.
  - The tiny weight matrices are replicated; per layer each core computes
    g = dis * h for its shard, the shards are AllGathered into a shared HBM
    table G, and each core aggregates messages for its own nodes with
    dma_gather (256B rows from G) + dma_scatter_add (CCE-add, SBUF
    destination in the parity-split layout).
  - Symmetric normalization is folded per-node:
       (Ahat h)[c] = dis[c] * sum_{e->c, self} dis[r] h[r]
    (self-loops are regular tokens) so no per-edge multiplies on-device.
  - Scatter rounds have distinct destination slots within each call
    (race-free CCE accumulation); a per-dest phase staggers rounds so each
    carries ~tot/cap tokens, and each round issues one scatter per
    dest-tile half (disjoint slot windows, and each call stays under the
    SWDGE descriptor-ring capacity) plus NC gathers (one per 32768-row
    int16 index window).  High-degree dests spill to a second virtual
    level; parity == level, so the readback is two contiguous A+B adds.
  - The LAST conv has a 1-wide output, so h@W3 is computed during the
    second layer's matmul tail and only the per-node scalars are exchanged
    (dense AllGather, 64x less traffic, expanded into column 0 of the
    table rows); the aggregation gathers the same 256B rows but scatters
    4B tokens.
  - Index tables are uploaded 16-wide (the native wrap) and replicated to
    the 128-partition layout once on device; x is uploaded in bf16.
"""
import sys
import numpy as np

sys.path.insert(0, "/opt/trn_rl_repo")

F = 64           # feature width (STATE == HID == 64)
NCORES = 8


class Plan:
    """Static (compile-time) layout shared by all cores, plus per-core data.

    Node layout: local node l = t*128 + p  (SBUF partition p, tile t).
    Global gather-table row of node (core k, local l) = k*sh + l.
    Scatter range of local l: rng = t // half (dest-tile halves); slot
    within the range: s_local = (t % half)*2 + lev, scatter index =
    s_local*128 + p (trash at s_local = q_slots, one per parity).  The
    parity of s_local equals the level, so buffer A collects level 0 and
    buffer B level 1, both at pair-column gpr*rng + (t % half), and the
    readback is acc[:, rng-half] = A[cols] + B[cols] per range.
    """

    def __init__(self, n_nodes, edge_index, row_chunk=32768):
        self.n_nodes = n_nodes
        self.row_chunk = row_chunk
        shard = (n_nodes + NCORES - 1) // NCORES
        sh = ((shard + 127) // 128) * 128
        self.shard, self.sh, self.t = shard, sh, sh // 128
        self.gtbl_rows = ((sh * NCORES + row_chunk - 1) // row_chunk) * row_chunk
        self.n_chunks = self.gtbl_rows // row_chunk

        loops = np.arange(n_nodes, dtype=np.int64)
        row = np.concatenate([np.asarray(edge_index[0], dtype=np.int64), loops])
        col = np.concatenate([np.asarray(edge_index[1], dtype=np.int64), loops])
        deg = np.bincount(col, minlength=n_nodes).astype(np.float64)
        self.dis = (1.0 / np.sqrt(deg)).astype(np.float32)

        tpos_row = (row // shard) * sh + (row % shard)
        dst_core = col // shard
        cloc = col % shard

        per_core = []
        maxdeg = 0
        for k in range(NCORES):
            m = dst_core == k
            r_k, c_k = tpos_row[m], cloc[m]
            o = np.argsort(c_k, kind="stable")
            cs = c_k[o]
            if cs.size:
                starts = np.r_[True, cs[1:] != cs[:-1]]
                run_starts = np.flatnonzero(starts)
                rid = np.cumsum(starts) - 1
                occ_s = np.arange(cs.size) - run_starts[rid]
                occ = np.empty_like(occ_s)
                occ[o] = occ_s
                maxdeg = max(maxdeg, int(occ_s.max()) + 1)
            else:
                occ = np.zeros(0, np.int64)
            per_core.append((r_k, c_k, occ))

        T = self.t
        assert T % 2 == 0
        self.levels = 2
        self.half = T // 2              # dest tiles per scatter range
        self.q_slots = self.levels * self.half   # slots per range
        self.w_slots = self.q_slots + 2          # + one trash slot per parity
        self.gpr = self.w_slots // 2    # buffer-pair columns per range
        self.agg_groups = self.gpr * self.levels
        assert (self.w_slots - 1) * 128 + 127 <= 32767

        self.cap = max((maxdeg + self.levels - 1) // self.levels, 1)
        assert maxdeg <= self.cap * self.levels

        NC = self.n_chunks
        NL = 2                          # scatter ranges (dest-tile halves)
        cnt = np.zeros((NCORES, self.cap, NL, NC), np.int64)
        for k, (r_k, c_k, occ) in enumerate(per_core):
            # per-dest phase staggers the rounds so each round carries
            # ~tot/cap tokens instead of round 0 taking every dest
            rnd = (occ % self.cap + c_k % self.cap) % self.cap
            rng = (c_k >> 7) // self.half
            chk = r_k // row_chunk
            np.add.at(cnt[k], (rnd, rng, chk), 1)
        n_act = cnt.max(axis=0)
        n_rlc = np.maximum((n_act + 127) // 128 * 128, 128)
        # real tokens per bucket (worst core), quantized up to 16: the
        # gather's count register lets the ucode skip the trailing pad
        # (indices there are -1); SPMD-shared so it must be the max.
        self.n_act = np.minimum(np.maximum((n_act + 15) // 16 * 16, 16), n_rlc)
        self.n_rlc = n_rlc                                   # [cap, NL, NC]
        self.n_rl = self.n_rlc.sum(axis=2)                   # [cap, NL]
        self.max_nrl = int(self.n_rl.max())
        self.tot_tok = int(self.n_rl.sum())
        # one scatter call per (round, range); keep it under the SWDGE
        # descriptor-ring capacity (~1024 descs at num_idxs/8 per desc)
        assert self.max_nrl <= 8064, self.max_nrl

        base = np.concatenate([[0], np.cumsum(self.n_rl.reshape(-1))])[:-1]
        base_rl = base.reshape(self.cap, NL)
        off_rlc = np.zeros((self.cap, NL, NC), np.int64)
        for r in range(self.cap):
            for l in range(NL):
                o = 0
                for c in range(NC):
                    off_rlc[r, l, c] = o
                    o += int(n_rlc[r, l, c])
        self.base_rl, self.off_rlc = base_rl, off_rlc

        self.gidx = []
        self.sidx = []
        for k, (r_k, c_k, occ) in enumerate(per_core):
            rnd = ((occ % self.cap + c_k % self.cap) % self.cap).astype(np.int64)
            lev = (occ // self.cap).astype(np.int64)
            tt = c_k >> 7
            pp = c_k & 127
            rng = tt // self.half
            # parity == level, pair-column == tile-within-range
            s_local = (tt % self.half) * 2 + lev
            chk = r_k // row_chunk
            gflat = np.full(self.tot_tok, -1, np.int64)
            sflat = np.full(self.tot_tok, self.q_slots * 128, np.int64)
            key = (rnd * NL + rng) * NC + chk
            order = np.argsort(key, kind="stable")
            ks = key[order]
            if ks.size:
                starts = np.r_[True, ks[1:] != ks[:-1]]
                run_starts = np.flatnonzero(starts)
                rid = np.cumsum(starts) - 1
                within = np.arange(ks.size) - run_starts[rid]
                rr = ks // (NL * NC)
                ll = (ks // NC) % NL
                cc = ks % NC
                pos = base_rl[rr, ll] + off_rlc[rr, ll, cc] + within
                e = order
                gflat[pos] = r_k[e] - cc * row_chunk
                sflat[pos] = s_local[e] * 128 + pp[e]
            # dummy row-0 gathers up to the shared count register value,
            # skipped (-1) beyond it
            for r in range(self.cap):
                for l in range(NL):
                    for c in range(NC):
                        s0 = base_rl[r, l] + off_rlc[r, l, c]
                        seg = gflat[s0:s0 + int(self.n_act[r, l, c])]
                        seg[seg < 0] = 0
            self.gidx.append(self._wrap(gflat))
            self.sidx.append(self._wrap(sflat))

        # int16-slot offsets into the flat [16, slots] index tables
        self.gso = {}
        off = 0
        for r in range(self.cap):
            for l in range(NL):
                for c in range(NC):
                    self.gso[(r, l, c)] = off
                    off += int(n_rlc[r, l, c]) // 16
        self.gslots = off
        self.sso = {}
        off = 0
        for r in range(self.cap):
            for l in range(NL):
                self.sso[(r, l)] = off
                off += int(self.n_rl[r, l]) // 16
        self.sslots = off

    @staticmethod
    def _wrap(idx):
        n = idx.size
        a = idx.astype(np.int16).reshape(n // 16, 16).T
        return np.ascontiguousarray(a)          # [16, n // 16]

    def core_inputs(self, k, x, W1, b1, W2, b2, W3, b3):
        import ml_dtypes
        sh, shard, t = self.sh, self.shard, self.t
        xs = np.zeros((sh, F), np.float32)
        lo, hi = k * shard, min((k + 1) * shard, self.n_nodes)
        xs[: hi - lo] = x[lo:hi]
        ds = np.zeros(sh, np.float32)
        ds[: hi - lo] = self.dis[lo:hi]
        x_dev = xs.reshape(t, 128, F).transpose(1, 0, 2)   # node l = t*128+p
        d_dev = ds.reshape(t, 128).T
        return {
            "x": np.ascontiguousarray(
                x_dev.reshape(128, t * F).astype(ml_dtypes.bfloat16)),
            "dis": np.ascontiguousarray(d_dev),
            "dis_flat": np.ascontiguousarray(ds.reshape(1, sh)),
            "gidx": self.gidx[k],
            "sidx": self.sidx[k],
            "W1": np.asarray(W1, np.float32),
            "b1": np.asarray(b1, np.float32).reshape(F, 1),
            "W2": np.asarray(W2, np.float32),
            "b2": np.asarray(b2, np.float32).reshape(F, 1),
            "W3": np.asarray(W3, np.float32).reshape(F, 1),
            "b3": np.asarray(b3, np.float32).reshape(1, 1),
        }

    def assemble(self, outs, b3=0.0):
        """outs: per core {'out': [sh]} with flat index == node local id."""
        res = np.zeros((self.n_nodes, 1), np.float32)
        for k in range(NCORES):
            o = np.asarray(outs[k]["out"]).reshape(self.sh)
            lo = k * self.shard
            hi = min(lo + self.shard, self.n_nodes)
            res[lo:hi, 0] = o[: hi - lo]
        return res + np.float32(b3).reshape(1, 1)


def build(plan, n_queues=4, scalar_l3=True):
    import concourse.bacc as bacc
    import concourse.mybir as mybir
    import concourse.tile as tile
    from concourse.masks import make_identity

    f32 = mybir.dt.float32
    bf16 = mybir.dt.bfloat16
    i16 = mybir.dt.int16
    T, SH = plan.t, plan.sh
    CAP, NC, NL = plan.cap, plan.n_chunks, plan.levels
    MAXNR = plan.max_nrl
    GPR = plan.gpr                     # pair-columns per range
    AGG = plan.agg_groups              # total pair-columns (NL * GPR)
    H = T // 2                         # even (= odd) tile count per range

    nc = bacc.Bacc("TRN2", target_bir_lowering=False, debug=False,
                   num_devices=NCORES, num_swdge_queues=n_queues)

    x_t = nc.dram_tensor("x", [128, T * F], bf16, kind="ExternalInput")
    dis_t = nc.dram_tensor("dis", [128, T], f32, kind="ExternalInput")
    disf_t = nc.dram_tensor("dis_flat", [1, SH], f32, kind="ExternalInput")
    gidx_t = nc.dram_tensor("gidx", [16, plan.gslots], i16, kind="ExternalInput")
    sidx_t = nc.dram_tensor("sidx", [16, plan.sslots], i16, kind="ExternalInput")
    Ws = {}
    for nm, shape in [("W1", [F, F]), ("b1", [F, 1]), ("W2", [F, F]),
                      ("b2", [F, 1]), ("W3", [F, 1]), ("b3", [1, 1])]:
        Ws[nm] = nc.dram_tensor(nm, shape, f32, kind="ExternalInput")
    out_t = nc.dram_tensor("out", [SH], f32, kind="ExternalOutput")

    gidx_r = nc.dram_tensor("gidx_rep", [128, plan.gslots], i16, kind="Internal")
    sidx_r = nc.dram_tensor("sidx_rep", [128, plan.sslots], i16, kind="Internal")
    g_dram = nc.dram_tensor("g_bounce", [SH * F], f32, kind="Internal")
    t3_dram = nc.dram_tensor("t3_bounce", [SH], f32, kind="Internal")
    T3d = nc.dram_tensor("T3_dense", [SH * NCORES], f32, kind="Internal",
                         addr_space="Shared")
    G = nc.dram_tensor("G_table", [plan.gtbl_rows, F], f32, kind="Internal",
                       addr_space="Shared")
    rg = [list(range(NCORES))]

    with tile.TileContext(nc) as tc:
        with tc.tile_pool(name="const", bufs=1) as cpool, \
             tc.tile_pool(name="state", bufs=1) as spool, \
             tc.tile_pool(name="agg", bufs=1) as apool, \
             tc.tile_pool(name="msg", bufs=4) as mpool, \
             tc.tile_pool(name="idx", bufs=4) as ipool, \
             tc.tile_pool(name="fm", bufs=2) as fpool, \
             tc.tile_pool(name="psum", bufs=2, space="PSUM") as ppool:

            # replicate the 16-wide index uploads to the 128-partition wrap
            # (on the otherwise-idle Activation engine so the SP-engine
            # bounce/collective chain is not queued behind them)
            for k in range(NCORES):
                nc.scalar.dma_start(gidx_r[16 * k:16 * (k + 1), :], gidx_t[:])
                nc.scalar.dma_start(sidx_r[16 * k:16 * (k + 1), :], sidx_t[:])

            # issue the x-load -> scale -> bounce chain FIRST: the SP queue
            # is in-order, so anything issued before it delays the first
            # collective; everything else below is needed much later
            dis_s = cpool.tile([128, T], f32)
            nc.sync.dma_start(dis_s[:], dis_t[:])
            dis_b = dis_s[:].unsqueeze(-1).broadcast_to([128, T, F])

            aggA = apool.tile([128, AGG, F], f32, tag="aggA")
            aggB = apool.tile([128, AGG, F], f32, tag="aggB")
            agg3A = apool.tile([128, AGG, 1], f32, tag="agg3A")
            agg3B = apool.tile([128, AGG, 1], f32, tag="agg3B")

            qctr = [0]

            def next_q():
                q = qctr[0] % n_queues
                qctr[0] += 1
                return q

            # the gather/scatter token counts are 128-quantized, so a few
            # shared registers replace one RegisterMove per call (~30us of
            # Pool-engine time per layer otherwise)
            reg_cache = {}

            def cached_reg(v):
                if v not in reg_cache:
                    reg_cache[v] = nc.gpsimd.to_reg(v)
                return reg_cache[v]

            xb = mpool.tile([128, T, F], bf16, tag="msg")
            nc.sync.dma_start(xb[:], x_t[:].rearrange("p (t f) -> p t f", f=F))
            g = spool.tile([128, T, F], f32, tag="g")
            nc.vector.tensor_tensor(g[:], xb[:], dis_b, mybir.AluOpType.mult)

            ident = cpool.tile([128, 128], f32)
            make_identity(nc, ident[:])
            wsb = {}
            for nm in ("W1", "W2", "W3", "b1", "b2", "b3"):
                wsb[nm] = cpool.tile(list(Ws[nm].shape), f32, name=f"sb_{nm}")
                nc.scalar.dma_start(wsb[nm][:], Ws[nm][:])

            # strided view: DRAM row l = t*128+p  <->  g[p, t, :]
            g_dram_v = g_dram[:].rearrange("(t p f) -> p t f", p=128, f=F)

            def do_rounds(scalar):
                for r in range(CAP):
                    for l in range(NL):
                        n_rl = int(plan.n_rl[r, l])
                        gi = ipool.tile([128, MAXNR // 16], i16, tag="gi")
                        si = ipool.tile([128, MAXNR // 16], i16, tag="si")
                        g0 = plan.gso[(r, l, 0)]
                        s0 = plan.sso[(r, l)]
                        nc.sync.dma_start(gi[:, : n_rl // 16],
                                          gidx_r[:, g0:g0 + n_rl // 16])
                        nc.sync.dma_start(si[:, : n_rl // 16],
                                          sidx_r[:, s0:s0 + n_rl // 16])
                        buf = mpool.tile([128, MAXNR // 128, F], f32, tag="msg")
                        for c in range(NC):
                            n_rlc = int(plan.n_rlc[r, l, c])
                            oc = int(plan.off_rlc[r, l, c])
                            go = plan.gso[(r, l, c)] - g0
                            hi = min((c + 1) * plan.row_chunk, SH * NCORES)
                            nc.gpsimd.dma_gather(
                                buf[:, oc // 128:(oc + n_rlc) // 128, :],
                                G[c * plan.row_chunk:hi, :],
                                gi[:, go:go + n_rlc // 16],
                                n_rlc, cached_reg(int(plan.n_act[r, l, c])), F,
                                queue_num=next_q(),
                                single_packet=bool(n_rlc <= 1024))
                        if scalar:
                            v = mpool.tile([128, MAXNR // 128, 1], f32,
                                           tag="vmsg")
                            nc.vector.tensor_copy(v[:, : n_rl // 128, 0:1],
                                                  buf[:, : n_rl // 128, 0:1])
                            nc.gpsimd.dma_scatter_add(
                                agg3A[:, GPR * l:GPR * (l + 1), :],
                                v[:, : n_rl // 128, :],
                                si[:, : n_rl // 16],
                                n_rl, cached_reg(n_rl), 1,
                                sbuf_tokens_per_rank=128,
                                parity_reg=cached_reg(0),
                                out_ap_other=agg3B[:, GPR * l:GPR * (l + 1), :],
                                queue_num=next_q())
                        else:
                            nc.gpsimd.dma_scatter_add(
                                aggA[:, GPR * l:GPR * (l + 1), :],
                                buf[:, : n_rl // 128, :],
                                si[:, : n_rl // 16],
                                n_rl, cached_reg(n_rl), F,
                                sbuf_tokens_per_rank=128,
                                parity_reg=cached_reg(0),
                                out_ap_other=aggB[:, GPR * l:GPR * (l + 1), :],
                                queue_num=next_q())

            n_floop = 2 if scalar_l3 else 3
            for layer in range(n_floop):
                if layer == 0:
                    nc.sync.dma_start(g_dram_v, g[:])
                # (later layers: the previous tail already wrote g_dram
                # block by block, so the collective can start immediately)
                nc.gpsimd.collective_compute(
                    "AllGather", mybir.AluOpType.bypass,
                    replica_groups=rg,
                    ins=[g_dram[:]],
                    outs=[G[0:SH * NCORES, :].rearrange("r f -> (r f)")],
                )
                # layer 0: the scheduler would front-run these on DVE ahead
                # of the critical g-scale; Pool is idle before the first
                # collective, so zero the buffers there instead
                meng = nc.gpsimd if layer == 0 else nc.vector
                meng.memset(aggA[:], 0.0)
                meng.memset(aggB[:], 0.0)

                do_rounds(scalar=False)

                # acc[tile j*H+u] = (A[range j, col u] + B[range j, col u])*dis,
                # per range so each half unblocks its matmul blocks early
                acc = spool.tile([128, T, F], f32, tag="acc")
                for j in range(NL):
                    sl = slice(j * H, (j + 1) * H)
                    nc.vector.tensor_tensor(
                        acc[:, sl, :],
                        aggA[:, GPR * j:GPR * j + H, :],
                        aggB[:, GPR * j:GPR * j + H, :], mybir.AluOpType.add)
                    nc.vector.tensor_tensor(acc[:, sl, :], acc[:, sl, :],
                                            dis_b[:, sl, :], mybir.AluOpType.mult)

                if not scalar_l3 and layer == 2:
                    W3, b3s = wsb["W3"], wsb["b3"]
                    for t0 in range(0, T, 4):
                        nt = min(4, T - t0)
                        fm = fpool.tile([F, 4 * 128], f32, tag="fm")
                        for j in range(nt):
                            pt = ppool.tile([F, 128], f32, space="PSUM", tag="pt")
                            nc.tensor.transpose(pt[:], acc[:, t0 + j, :], ident[:])
                            nc.vector.tensor_copy(fm[:, j * 128:(j + 1) * 128], pt[:])
                        mm3 = ppool.tile([1, 4 * 128], f32, space="PSUM", tag="mm3")
                        nc.tensor.matmul(mm3[:, : nt * 128], W3[:], fm[:, : nt * 128],
                                         start=True, stop=True)
                        ofm = fpool.tile([1, 4 * 128], f32, tag="tofm")
                        nc.vector.tensor_scalar_add(
                            ofm[:, : nt * 128], mm3[:, : nt * 128], b3s[:, :1])
                        nc.sync.dma_start(
                            out_t[t0 * 128:(t0 + nt) * 128]
                            .rearrange("(a x) -> a x", a=1),
                            ofm[:, : nt * 128])
                    continue
                W, b = wsb[f"W{layer + 1}"], wsb[f"b{layer + 1}"]
                last = layer == 1 and scalar_l3
                if last:
                    W3 = wsb["W3"]
                for t0 in range(0, T, 4):
                    nt = min(4, T - t0)
                    ptA = ppool.tile([F, 4 * 128], f32, space="PSUM", tag="pt")
                    for j in range(nt):
                        nc.tensor.transpose(ptA[:, j * 128:(j + 1) * 128],
                                            acc[:, t0 + j, :], ident[:])
                    fm = fpool.tile([F, 4 * 128], f32, tag="fm")
                    nc.vector.tensor_copy(fm[:, : nt * 128], ptA[:, : nt * 128])
                    mm = ppool.tile([F, 4 * 128], f32, space="PSUM", tag="mm")
                    nc.tensor.matmul(mm[:, : nt * 128], W[:], fm[:, : nt * 128],
                                     start=True, stop=True)
                    hfm = fpool.tile([F, 4 * 128], f32, tag="hfm")
                    nc.scalar.activation(hfm[:, : nt * 128], mm[:, : nt * 128],
                                         mybir.ActivationFunctionType.Relu,
                                         bias=b[:, :1])
                    if not last:
                        # transpose back, scale by next-layer dis, and ship
                        # the block straight to the bounce buffer
                        pt2 = ppool.tile([128, 4 * F], f32, space="PSUM",
                                         tag="pt2")
                        for j in range(nt):
                            nc.tensor.transpose(
                                pt2[:, j * F:(j + 1) * F],
                                hfm[:, j * 128:(j + 1) * 128], ident[:F, :F])
                        gblk = fpool.tile([128, 4 * F], f32, tag="gblk")
                        gblk_v = gblk[:].rearrange("p (t f) -> p t f", f=F)
                        pt2_v = pt2[:].rearrange("p (t f) -> p t f", f=F)
                        nc.vector.tensor_tensor(
                            gblk_v[:, : nt, :], pt2_v[:, : nt, :],
                            dis_b[:, t0:t0 + nt, :], mybir.AluOpType.mult)
                        nc.sync.dma_start(g_dram_v[:, t0:t0 + nt, :],
                                          gblk_v[:, : nt, :])
                    else:
                        # z = h2 @ W3 (scalar per node), t = dis * z,
                        # written straight into column 0 of the table rows
                        mm3 = ppool.tile([1, 4 * 128], f32, space="PSUM", tag="mm3")
                        nc.tensor.matmul(mm3[:, : nt * 128], W3[:],
                                         hfm[:, : nt * 128], start=True, stop=True)
                        df = fpool.tile([1, 4 * 128], f32, tag="df")
                        nc.scalar.dma_start(
                            df[:, : nt * 128],
                            disf_t[:, t0 * 128:t0 * 128 + nt * 128])
                        tofm = fpool.tile([1, 4 * 128], f32, tag="tofm")
                        nc.vector.tensor_tensor(
                            tofm[:, : nt * 128], mm3[:, : nt * 128],
                            df[:, : nt * 128], mybir.AluOpType.mult)
                        nc.sync.dma_start(
                            t3_dram[t0 * 128:t0 * 128 + nt * 128]
                            .rearrange("(a x) -> a x", a=1),
                            tofm[:, : nt * 128])

            # third conv: exchange the per-node scalars densely (64x less
            # traffic than the F-wide table), expand into column 0 of the
            # table rows, gather the same 256B rows, scatter 4B tokens.
            if scalar_l3:
                nc.gpsimd.collective_compute(
                    "AllGather", mybir.AluOpType.bypass,
                    replica_groups=rg,
                    ins=[t3_dram[:]],
                    outs=[T3d[:]],
                )
                tsb = spool.tile([128, NCORES * T], f32, tag="tsb")
                nc.sync.dma_start(tsb[:], T3d[:].rearrange("(c p) -> p c", p=128))
                nc.sync.dma_start(
                    G[0:SH * NCORES, :]
                    .rearrange("(c p) f -> p c f", p=128)[:, :, 0:1],
                    tsb[:].unsqueeze(-1))
                nc.vector.memset(agg3A[:], 0.0)
                nc.vector.memset(agg3B[:], 0.0)

                do_rounds(scalar=True)

                res = spool.tile([128, T], f32, tag="res")
                for j in range(NL):
                    nc.vector.tensor_tensor(
                        res[:, j * H:(j + 1) * H],
                        agg3A[:, GPR * j:GPR * j + H, 0],
                        agg3B[:, GPR * j:GPR * j + H, 0], mybir.AluOpType.add)
                nc.vector.tensor_tensor(res[:], res[:], dis_s[:],
                                        mybir.AluOpType.mult)
                nc.sync.dma_start(out_t[:].rearrange("(t p) -> p t", p=128),
                                  res[:])

    nc.compile()
    return nc


def kernel(**inputs):
    from concourse import bass2jax

    x = np.asarray(inputs["x"], np.float32)
    edge_index = np.asarray(inputs["edge_index"])
    plan = Plan(x.shape[0], edge_index)
    nc = build(plan)
    in_maps = [plan.core_inputs(k, x, inputs["W1"], inputs["b1"], inputs["W2"],
                                inputs["b2"], inputs["W3"], inputs["b3"])
               for k in range(NCORES)]
    results = bass2jax.run_bass_via_pjrt(nc, in_maps, n_cores=NCORES)
    return plan.assemble(results, b3=np.asarray(inputs["b3"]).reshape(-1)[0])
